# Initial kernel scaffold
#
"""Trainium2 Bass kernel for nn_CameraViewTransformerLSS (LSS camera->BEV transformer).

Pipeline (B=1, N=6 cams, D=48 depth bins, 64x176 feature map, C=80 ctx channels,
128x128 BEV grid, 128 output channels):

  1. lift:    feat[n,d,h,w,c] = depth_prob[n,d,h,w] * context[n,c,h,w]
  2. splat:   scatter-add feat into BEV bins by frustum geometry
  3. head:    1x1 conv (80->128) + BN + ReLU

Key structural fact: with this camera rig (rotations about z only), the BEV bin
of a frustum point depends only on (camera n, depth d, image column w) -- NOT on
the image row h.  So the h axis can be contracted *before* any scatter:

  partial[(n,w,d), c] = sum_h depth[n,d,h,w] * ctx[n,c,h,w]     (a small matmul
  per camera-column "ray", K=h=64), reducing the scatter from 3.24M points to
  50688 points.

Both launches are DMA-transfer bound (all DMA transfers serialize on the
device's DMA engine pool at ~360 GB/s), so the layouts minimize bytes moved:

  L1 (ray-sharded lift): depth (48 cols) and ctx (80 cols) of a ray pair are
      packed side by side into one 128-col block -> zero padding, one DMA
      stream.  Outputs are packed into 96 PSUM partitions (2 rays x 48 d) so
      the partial write is exactly 132 rays x 48 d x 80 c bf16 per core.
  host (free): sort the valid partial rows by BEV bin into padded 128-point
      K-tiles; snake-balance BEV rows across cores (16 rows each, uniform tile
      schedule so all cores run the identical program); fold BN scale into the
      1x1 conv weight.
  L2 (bin-sharded scatter + head): each core scatter-accumulates its K-tiles
      (80-wide raw partials) into PSUM with one-hot matmuls (one-hot built
      on-device: iota == idx), 4 BEV rows per PSUM bank; then a 1x1-conv
      matmul (80->128, folded BN scale) per bank, BN-bias + ReLU straight out
      of PSUM, and a bf16 output write.

The bin indices are computed on host with jnp mirroring the reference op
sequence exactly (a few points land exactly on bin boundaries; same backend =>
identical floor results).
"""

import functools

import numpy as np

import concourse.bacc as bacc
import concourse.mybir as mybir
import concourse.tile as tile
from concourse.bass_utils import run_bass_kernel_spmd

# ---------------------------------------------------------------- constants
NCAM, DD, HF, WF, CC = 6, 48, 64, 176, 80
BH = BW = 128
OC = 128
STRIDE = 4.0
PC = (-50.0, -50.0, -5.0, 50.0, 50.0, 3.0)
Z_MIN, Z_MAX = 1.0, 60.0
BN_EPS = 1e-5

NCORES = 8
RAYS = NCAM * WF            # 1056
RPC = RAYS // NCORES        # 132 rays per core
GPC = RPC // 4              # 33 groups of 4 rays
NSLOT = BH // NCORES        # 16 BEV rows per core
F32 = mybir.dt.float32
BF16 = mybir.dt.bfloat16

PAIR = DD + CC              # 128: depth cols + ctx cols of one ray


def _np_bf16():
    import ml_dtypes

    return np.dtype(ml_dtypes.bfloat16)


# ---------------------------------------------------------------- L1 builder
@functools.lru_cache(maxsize=2)
def _build_l1():
    nc = bacc.Bacc("TRN2", target_bir_lowering=False, debug=False, num_devices=NCORES)
    # per ray pair j: cols [j*128, j*128+48) = depth, [j*128+48, (j+1)*128) = ctx
    # partitions 0:64 = h of first ray, 64:128 = h of second ray
    dc_in = nc.dram_tensor("dc_in", [128, GPC * 2 * PAIR], BF16, kind="ExternalInput")
    part = nc.dram_tensor("part", [112, GPC * 2 * CC], BF16, kind="ExternalOutput")

    CHS = (2, 4, 9, 9, 9)        # groups per input chunk (small first chunk
    BK = 512                     # so compute starts early)
    OGR = 6                      # groups per output DMA chunk

    with tile.TileContext(nc) as tc:
        with (
            tc.tile_pool(name="dc", bufs=4) as dc_pool,
            tc.tile_pool(name="stage", bufs=1) as stage_pool,
            tc.tile_pool(name="ps", bufs=2, space="PSUM") as ps_pool,
            tc.tile_pool(name="warm", bufs=1) as warm_pool,
        ):
            # prewarm the Activation function table (1.28us load) at t=0 so
            # it is off the critical path of the first real Act copy
            warm = warm_pool.tile([128, 2], BF16)
            nc.gpsimd.memset(warm[:, 0:1], 0)
            nc.scalar.activation(
                out=warm[:, 1:2], in_=warm[:, 0:1],
                func=mybir.ActivationFunctionType.Relu,
            )
            stage = stage_pool.tile([112, GPC * 160], BF16)
            g0 = 0
            for ch, GCH in enumerate(CHS):
                dct = dc_pool.tile([128, GCH * 2 * PAIR], BF16, tag="dc")
                deng = nc.sync
                deng.dma_start(
                    out=dct[:], in_=dc_in[:, g0 * 2 * PAIR:(g0 + GCH) * 2 * PAIR]
                )
                for gg in range(GCH):
                    g = g0 + gg
                    # 4-bank PSUM tile shared by 2 groups (4 ray pairs); one
                    # wide copy per 2 groups halves the copy init overhead.
                    if g % 2 == 0:
                        pt = ps_pool.tile([128, 4 * BK], F32, space="PSUM")
                    for pj in range(2):            # pair index within group
                        j = 2 * gg + pj
                        bank = 2 * (g % 2) + pj
                        dsl = slice(j * PAIR, j * PAIR + DD)
                        csl = slice(j * PAIR + DD, (j + 1) * PAIR)
                        # ray 4g+pj (PE rows 0:64) -> partitions 0:48.
                        # lhsT widened to 64 cols (16 ctx cols as junk
                        # weights) so PSUM rows 48:64 are initialized for the
                        # block copy; rows 48:64 are dropped at host unpack.
                        nc.tensor.matmul(
                            out=pt[0:64, bank * BK:bank * BK + CC],
                            lhsT=dct[0:64, slice(j * PAIR, j * PAIR + 64)],
                            rhs=dct[0:64, csl],
                            start=True,
                            stop=True,
                        )
                        # ray 4g+2+pj (PE rows 64:128) -> partitions 48:96
                        nc.tensor.matmul(
                            out=pt[64:64 + DD, bank * BK:bank * BK + CC],
                            lhsT=dct[64:128, dsl],
                            rhs=dct[64:128, csl],
                            start=True,
                            stop=True,
                        )
                    if g % 2 == 1 or g == GPC - 1:
                        nb = 2 * (g % 2) + 2        # banks filled in this tile
                        blo = (g // 2) * 2          # first group in the tile
                        src = pt[0:112, 0:nb * BK].rearrange(
                            "p (b x) -> p b x", b=nb
                        )[:, :, 0:CC]
                        dst = stage[:, blo * 160:(g + 1) * 160].rearrange(
                            "p (b x) -> p b x", b=nb
                        )
                        if (g // 2) % 2 == 0:
                            nc.scalar.copy(out=dst, in_=src)
                        else:
                            nc.vector.tensor_copy(out=dst, in_=src)
                    # stream partials out on the idle GPSIMD SWDGE queue
                    # (final chunk on HWDGE: no ~1us SWDGE desc-gen in tail)
                    if (g + 1) % OGR == 0 or g == GPC - 1:
                        lo = (g // OGR) * OGR * 160
                        hi = (g + 1) * 160
                        oeng = nc.sync if g == GPC - 1 else nc.gpsimd
                        oeng.dma_start(out=part[:, lo:hi], in_=stage[:, lo:hi])
                g0 += GCH
    nc.compile()
    return nc


# Unpack map for L1 "part" output (112 partitions; rows 48:64 are junk):
#   parts 0:48   : [:, g, 0] = ray 4g+0, [:, g, 1] = ray 4g+1
#   parts 64:112 : [:, g, 0] = ray 4g+2, [:, g, 1] = ray 4g+3
def _unpack_l1(out_core):
    S = out_core.reshape(112, GPC, 2, CC)
    p = np.empty((RPC, DD, CC), out_core.dtype)
    p[0::4] = S[0:DD, :, 0].transpose(1, 0, 2)
    p[1::4] = S[0:DD, :, 1].transpose(1, 0, 2)
    p[2::4] = S[64:112, :, 0].transpose(1, 0, 2)
    p[3::4] = S[64:112, :, 1].transpose(1, 0, 2)
    return p


# ---------------------------------------------------------------- L2 builder
@functools.lru_cache(maxsize=8)
def _build_l2(K):
    """K: tuple of NSLOT ints -- tiles per PSUM row-slot (uniform across cores).

    Slots are grouped 4 to a PSUM bank (4 BEV rows = 512 fp32 columns); after a
    bank's 4 slots finish accumulating 80-wide partial sums, the bank is copied
    to SBUF and hit with one 1x1-conv matmul (80->128, BN scale folded into the
    weight on host), then BN-bias + ReLU straight out of conv PSUM into the
    bf16 output stage.
    """
    T_u = sum(K)
    nc = bacc.Bacc("TRN2", target_bir_lowering=False, debug=False, num_devices=NCORES)
    vals = nc.dram_tensor("vals", [128, T_u * CC], BF16, kind="ExternalInput")
    # merged metadata: cols 0:64 = iota as raw bf16 pairs, 64:64+T_u = idx,
    # col 64+T_u = BN bias, cols 65+T_u:129+T_u = folded conv weight (bf16 raw,
    # partitions 0:80)
    meta = nc.dram_tensor("meta", [128, 129 + T_u], F32, kind="ExternalInput")
    y = nc.dram_tensor("y", [OC, NSLOT * BW], BF16, kind="ExternalOutput")

    b1 = min(K[0], T_u)          # small first chunk so compute starts early
    rest = T_u - b1
    bnd = [0, b1, b1 + rest // 3, b1 + (2 * rest) // 3, T_u]     # tile chunks

    with tile.TileContext(nc) as tc:
        with (
            tc.tile_pool(name="consts", bufs=1) as const_pool,
            tc.tile_pool(name="vals", bufs=4) as vals_pool,
            tc.tile_pool(name="oh", bufs=20) as oh_pool,
            tc.tile_pool(name="bev", bufs=2) as bev_pool,
            tc.tile_pool(name="yst", bufs=1) as yst_pool,
            tc.tile_pool(name="psA", bufs=2, space="PSUM") as psA_pool,
            tc.tile_pool(name="psC", bufs=2, space="PSUM") as psC_pool,
        ):
            # prewarm the Activation function table at t=0
            warm = const_pool.tile([128, 2], BF16)
            nc.gpsimd.memset(warm[:, 0:1], 0)
            nc.scalar.activation(
                out=warm[:, 1:2], in_=warm[:, 0:1],
                func=mybir.ActivationFunctionType.Relu,
            )
            meta_t = const_pool.tile([128, 129 + T_u], F32)
            nc.sync.dma_start(out=meta_t[:], in_=meta[:])
            iota_t = meta_t[:, 0:64].bitcast(BF16)          # (128, 128) iota
            idx_t = meta_t[:, 64:64 + T_u]
            bias_ap = meta_t[:, 64 + T_u:65 + T_u]
            wS_t = meta_t[0:CC, 65 + T_u:129 + T_u].bitcast(BF16)   # (80, 128)

            vt = []
            for chk in range(4):
                t = vals_pool.tile([128, (bnd[chk + 1] - bnd[chk]) * CC], BF16)
                eng = nc.sync
                eng.dma_start(
                    out=t[:], in_=vals[:, bnd[chk] * CC:bnd[chk + 1] * CC]
                )
                vt.append(t)

            def val_slice(tf):
                chk = next(i for i in range(4) if bnd[i] <= tf < bnd[i + 1])
                lo = (tf - bnd[chk]) * CC
                return vt[chk][:, lo:lo + CC]

            yst = yst_pool.tile([OC, NSLOT * BW], BF16)
            tf = 0
            for q in range(4):                   # 4 slots -> 1 PSUM bank
                ps = psA_pool.tile([128, 512], F32, space="PSUM")
                for i in range(4):
                    s = 4 * q + i
                    for k in range(K[s]):
                        oh = oh_pool.tile([128, 128], BF16)
                        # spread some one-hot builds onto the GPSIMD engine
                        oheng = nc.gpsimd if (tf % 3 == 2) else nc.vector
                        oheng.tensor_scalar(
                            out=oh[:],
                            in0=iota_t,
                            scalar1=idx_t[:, tf:tf + 1],
                            scalar2=None,
                            op0=mybir.AluOpType.is_equal,
                        )
                        nc.tensor.matmul(
                            out=ps[0:CC, i * BW:(i + 1) * BW],
                            lhsT=val_slice(tf),
                            rhs=oh[:],
                            start=(k == 0),
                            stop=(k == K[s] - 1),
                        )
                        tf += 1
                # drain bank: copy -> conv -> BN bias + ReLU, each stage on
                # its own engine so the 4 bank drains pipeline cleanly
                bev = bev_pool.tile([CC, 512], BF16)
                if q % 2 == 0:
                    nc.vector.tensor_copy(out=bev[:], in_=ps[0:CC, :])
                else:
                    nc.scalar.copy(out=bev[:], in_=ps[0:CC, :])
                psc = psC_pool.tile([OC, 512], F32, space="PSUM")
                nc.tensor.matmul(
                    out=psc[:], lhsT=wS_t, rhs=bev[:], start=True, stop=True
                )
                nc.scalar.activation(
                    out=yst[:, q * 512:(q + 1) * 512],
                    in_=psc[:],
                    func=mybir.ActivationFunctionType.Relu,
                    bias=bias_ap,
                    scale=1.0,
                )
                yeng = nc.sync
                yeng.dma_start(
                    out=y[:, q * 512:(q + 1) * 512],
                    in_=yst[:, q * 512:(q + 1) * 512],
                )
    nc.compile()
    return nc


# ---------------------------------------------------------------- host plan
def _compute_bins(intrinsics, cam2ego):
    """Mirror the reference's index math exactly (same jnp ops, same backend)
    so floor() results match bit-for-bit, then reduce over the h axis."""
    import jax.numpy as jnp

    intrinsics = jnp.asarray(intrinsics)
    cam2ego = jnp.asarray(cam2ego)
    u = ((jnp.arange(WF, dtype=jnp.float32) + 0.5) * STRIDE)[None, None, None, None, :]
    v = ((jnp.arange(HF, dtype=jnp.float32) + 0.5) * STRIDE)[None, None, None, :, None]
    Z = jnp.linspace(Z_MIN, Z_MAX, DD, dtype=jnp.float32)[None, None, :, None, None]

    fx = intrinsics[:, :, 0, 0][:, :, None, None, None]
    fy = intrinsics[:, :, 1, 1][:, :, None, None, None]
    cx = intrinsics[:, :, 0, 2][:, :, None, None, None]
    cy = intrinsics[:, :, 1, 2][:, :, None, None, None]

    Xc = (u - cx) / fx * Z
    Yc = (v - cy) / fy * Z
    Zc = jnp.broadcast_to(Z, Xc.shape)

    T = cam2ego[:, :, None, None, None]
    x_e = T[..., 0, 0] * Xc + T[..., 0, 1] * Yc + T[..., 0, 2] * Zc + T[..., 0, 3]
    y_e = T[..., 1, 0] * Xc + T[..., 1, 1] * Yc + T[..., 1, 2] * Zc + T[..., 1, 3]

    mx = (PC[3] - PC[0]) / BW
    my = (PC[4] - PC[1]) / BH
    ix = jnp.floor((x_e - PC[0]) / mx).astype(jnp.int32)
    iy = jnp.floor((y_e - PC[1]) / my).astype(jnp.int32)
    valid = (ix >= 0) & (ix < BW) & (iy >= 0) & (iy < BH)

    ix = np.asarray(ix)[0]
    iy = np.asarray(iy)[0]
    valid = np.asarray(valid)[0]
    # h-independence (holds for z-yaw-only rigs; required by this kernel)
    assert (ix == ix[:, :, :1, :]).all() and (iy == iy[:, :, :1, :]).all() and (
        valid == valid[:, :, :1, :]
    ).all(), "BEV bin depends on image row; kernel assumes z-yaw-only rig"
    return ix[:, :, 0, :], iy[:, :, 0, :], valid[:, :, 0, :]   # (N, D, W)


def _plan(intrinsics, cam2ego):
    ix, iy, valid = _compute_bins(intrinsics, cam2ego)
    # global point id = ray*DD + d, ray = n*WF + w
    ixr = ix.transpose(0, 2, 1).reshape(-1)      # (n, w, d) flattened
    iyr = iy.transpose(0, 2, 1).reshape(-1)
    vr = valid.transpose(0, 2, 1).reshape(-1)
    pid = np.arange(RAYS * DD, dtype=np.int64)

    vpid = pid[vr]
    vrow = iyr[vr].astype(np.int64)
    vcol = ixr[vr].astype(np.int64)

    # group points by BEV row
    order = np.argsort(vrow, kind="stable")
    vpid, vrow, vcol = vpid[order], vrow[order], vcol[order]
    rowcnt = np.bincount(vrow, minlength=BH)
    rowstart = np.concatenate([[0], np.cumsum(rowcnt)])
    tiles_per_row = np.maximum((rowcnt + 127) // 128, rowcnt > 0).astype(int)

    # snake-deal rows to cores by descending tile count -> 16 rows per core
    rorder = np.argsort(-tiles_per_row, kind="stable")
    core_rows = [[] for _ in range(NCORES)]
    for i, r in enumerate(rorder):
        rnd, pos = divmod(i, NCORES)
        c = pos if rnd % 2 == 0 else NCORES - 1 - pos
        core_rows[c].append(int(r))
    # per-core: rows sorted by tile count desc -> slot s
    for c in range(NCORES):
        core_rows[c].sort(key=lambda r: -tiles_per_row[r])
    K = tuple(
        int(max(tiles_per_row[core_rows[c][s]] for c in range(NCORES)))
        for s in range(NSLOT)
    )
    return dict(
        K=K,
        core_rows=core_rows,
        rowstart=rowstart,
        rowcnt=rowcnt,
        vpid=vpid,
        vcol=vcol,
    )


# ---------------------------------------------------------------- main entry
def _l1_inputs(depth_prob, context):
    dt = _np_bf16()
    dT = depth_prob[0].transpose(2, 0, 3, 1).reshape(HF, RAYS, DD)  # h, ray, d
    cT = context[0].transpose(2, 0, 3, 1).reshape(HF, RAYS, CC)     # h, ray, c
    maps = []
    for c in range(NCORES):
        sl = slice(c * RPC, (c + 1) * RPC)
        d4 = dT[:, sl].reshape(HF, GPC, 2, 2, DD)    # h, g, half, pj, d
        c4 = cT[:, sl].reshape(HF, GPC, 2, 2, CC)
        dc = np.concatenate([d4, c4], axis=-1)       # h, g, half, pj, 128
        dc = dc.transpose(2, 0, 1, 3, 4).reshape(128, GPC * 2 * PAIR)
        maps.append({"dc_in": np.ascontiguousarray(dc).astype(dt)})
    return maps


def _l2_inputs(plan, part_all, w_proj, b_proj, bn_gamma, bn_beta, bn_mean, bn_var):
    dt = _np_bf16()
    K = plan["K"]
    T_u = sum(K)
    scale = (bn_gamma / np.sqrt(bn_var + BN_EPS)).astype(np.float32)
    bias = ((b_proj - bn_mean) * scale + bn_beta).astype(np.float32)
    # fold BN scale into the conv weight; device conv runs after the scatter.
    wS = (w_proj * scale[:, None]).astype(np.float32)        # (OC, CC)
    wS_raw = np.zeros((128, 64), np.float32)
    wS_raw[0:CC] = np.ascontiguousarray(wS.T.astype(dt)).view(np.float32)
    iota_raw = np.ascontiguousarray(
        np.broadcast_to(np.arange(128, dtype=np.float32).astype(dt), (128, 128))
    ).view(np.float32)                                       # (128, 64) raw bits

    rowstart, vpid, vcol = plan["rowstart"], plan["vpid"], plan["vcol"]
    maps = []
    for c in range(NCORES):
        vals = np.zeros((128, T_u, CC), np.float32)
        idx = np.full((128, T_u), -1.0, np.float32)
        tf = 0
        for s in range(NSLOT):
            r = plan["core_rows"][c][s]
            lo, hi = rowstart[r], rowstart[r + 1]
            pids = vpid[lo:hi]
            cols = vcol[lo:hi]
            for k in range(K[s]):
                seg = slice(k * 128, min((k + 1) * 128, hi - lo))
                n = max(0, seg.stop - seg.start)
                if n > 0:
                    vals[:n, tf] = part_all[pids[seg]]
                    idx[:n, tf] = cols[seg]
                tf += 1
        meta = np.concatenate(
            [iota_raw, idx, bias[:, None], wS_raw], axis=1
        ).astype(np.float32)
        maps.append(
            {
                "vals": vals.reshape(128, -1).astype(dt),
                "meta": meta,
            }
        )
    return maps


def kernel(**inputs) -> np.ndarray:
    depth_prob = np.asarray(inputs["depth_prob"], np.float32)
    context = np.asarray(inputs["context"], np.float32)
    intrinsics = np.asarray(inputs["intrinsics"], np.float32)
    cam2ego = np.asarray(inputs["cam2ego"], np.float32)

    plan = _plan(intrinsics, cam2ego)
    nc1 = _build_l1()
    l1_maps = _l1_inputs(depth_prob, context)
    res1 = run_bass_kernel_spmd(nc1, l1_maps, list(range(NCORES))).results

    part_all = np.concatenate(
        [_unpack_l1(res1[c]["part"]) for c in range(NCORES)], axis=0
    ).reshape(RAYS * DD, CC).astype(np.float32)

    nc2 = _build_l2(plan["K"])
    l2_maps = _l2_inputs(
        plan,
        part_all,
        np.asarray(inputs["w_proj"], np.float32),
        np.asarray(inputs["b_proj"], np.float32),
        np.asarray(inputs["bn_gamma"], np.float32),
        np.asarray(inputs["bn_beta"], np.float32),
        np.asarray(inputs["bn_mean"], np.float32),
        np.asarray(inputs["bn_var"], np.float32),
    )
    res2 = run_bass_kernel_spmd(nc2, l2_maps, list(range(NCORES))).results

    y = np.empty((1, OC, BH, BW), np.float32)
    for c in range(NCORES):
        yc = np.asarray(res2[c]["y"], np.float32)      # (OC, NSLOT*BW)
        for s in range(NSLOT):
            r = plan["core_rows"][c][s]
            y[0, :, r, :] = yc[:, s * BW:(s + 1) * BW]
    return y



# revision 1
# speedup vs baseline: 1.0004x; 1.0004x over previous
"""Trainium2 Bass kernel for nn_CameraViewTransformerLSS (LSS camera->BEV transformer).

Pipeline (B=1, N=6 cams, D=48 depth bins, 64x176 feature map, C=80 ctx channels,
128x128 BEV grid, 128 output channels):

  1. lift:    feat[n,d,h,w,c] = depth_prob[n,d,h,w] * context[n,c,h,w]
  2. splat:   scatter-add feat into BEV bins by frustum geometry
  3. head:    1x1 conv (80->128) + BN + ReLU

Key structural fact: with this camera rig (rotations about z only), the BEV bin
of a frustum point depends only on (camera n, depth d, image column w) -- NOT on
the image row h.  So the h axis can be contracted *before* any scatter:

  partial[(n,w,d), c] = sum_h depth[n,d,h,w] * ctx[n,c,h,w]     (a small matmul
  per camera-column "ray", K=h=64), reducing the scatter from 3.24M points to
  50688 points.

Both launches are DMA-transfer bound (all DMA transfers serialize on the
device's DMA engine pool at ~360 GB/s), so the layouts minimize bytes moved:

  L1 (ray-sharded lift): depth (48 cols) and ctx (80 cols) of a ray pair are
      packed side by side into one 128-col block -> zero padding, one DMA
      stream.  Outputs are packed into 96 PSUM partitions (2 rays x 48 d) so
      the partial write is exactly 132 rays x 48 d x 80 c bf16 per core.
  host (free): sort the valid partial rows by BEV bin into padded 128-point
      K-tiles; snake-balance BEV rows across cores (16 rows each, uniform tile
      schedule so all cores run the identical program); fold BN scale into the
      1x1 conv weight.
  L2 (bin-sharded scatter + head): each core scatter-accumulates its K-tiles
      (80-wide raw partials) into PSUM with one-hot matmuls (one-hot built
      on-device: iota == idx), 4 BEV rows per PSUM bank; then a 1x1-conv
      matmul (80->128, folded BN scale) per bank, BN-bias + ReLU straight out
      of PSUM, and a bf16 output write.

The bin indices are computed on host with jnp mirroring the reference op
sequence exactly (a few points land exactly on bin boundaries; same backend =>
identical floor results).
"""

import functools

import numpy as np

import concourse.bacc as bacc
import concourse.mybir as mybir
import concourse.tile as tile
from concourse.bass_utils import run_bass_kernel_spmd

# ---------------------------------------------------------------- constants
NCAM, DD, HF, WF, CC = 6, 48, 64, 176, 80
BH = BW = 128
OC = 128
STRIDE = 4.0
PC = (-50.0, -50.0, -5.0, 50.0, 50.0, 3.0)
Z_MIN, Z_MAX = 1.0, 60.0
BN_EPS = 1e-5

NCORES = 8
RAYS = NCAM * WF            # 1056
RPC = RAYS // NCORES        # 132 rays per core
GPC = RPC // 4              # 33 groups of 4 rays
NSLOT = BH // NCORES        # 16 BEV rows per core
F32 = mybir.dt.float32
BF16 = mybir.dt.bfloat16

PAIR = DD + CC              # 128: depth cols + ctx cols of one ray


def _np_bf16():
    import ml_dtypes

    return np.dtype(ml_dtypes.bfloat16)


# ---------------------------------------------------------------- L1 builder
@functools.lru_cache(maxsize=2)
def _build_l1():
    nc = bacc.Bacc("TRN2", target_bir_lowering=False, debug=False, num_devices=NCORES)
    # per ray pair j: cols [j*128, j*128+48) = depth, [j*128+48, (j+1)*128) = ctx
    # partitions 0:64 = h of first ray, 64:128 = h of second ray
    dc_in = nc.dram_tensor("dc_in", [128, GPC * 2 * PAIR], BF16, kind="ExternalInput")
    part = nc.dram_tensor("part", [112, GPC * 2 * CC], BF16, kind="ExternalOutput")

    CHS = (2, 4, 9, 9, 9)        # groups per input chunk (small first chunk
    BK = 512                     # so compute starts early)
    OGR = 6                      # groups per output DMA chunk

    with tile.TileContext(nc) as tc:
        with (
            tc.tile_pool(name="dc", bufs=4) as dc_pool,
            tc.tile_pool(name="stage", bufs=1) as stage_pool,
            tc.tile_pool(name="ps", bufs=2, space="PSUM") as ps_pool,
            tc.tile_pool(name="warm", bufs=1) as warm_pool,
        ):
            # prewarm the Activation function table (1.28us load) at t=0 so
            # it is off the critical path of the first real Act copy
            warm = warm_pool.tile([128, 2], BF16)
            nc.gpsimd.memset(warm[:, 0:1], 0)
            nc.scalar.activation(
                out=warm[:, 1:2], in_=warm[:, 0:1],
                func=mybir.ActivationFunctionType.Relu,
            )
            stage = stage_pool.tile([112, GPC * 160], BF16)
            g0 = 0
            for ch, GCH in enumerate(CHS):
                dct = dc_pool.tile([128, GCH * 2 * PAIR], BF16, tag="dc")
                deng = nc.sync
                deng.dma_start(
                    out=dct[:], in_=dc_in[:, g0 * 2 * PAIR:(g0 + GCH) * 2 * PAIR]
                )
                for gg in range(GCH):
                    g = g0 + gg
                    # 4-bank PSUM tile shared by 2 groups (4 ray pairs); one
                    # wide copy per 2 groups halves the copy init overhead.
                    if g % 2 == 0:
                        pt = ps_pool.tile([128, 4 * BK], F32, space="PSUM")
                    for pj in range(2):            # pair index within group
                        j = 2 * gg + pj
                        bank = 2 * (g % 2) + pj
                        dsl = slice(j * PAIR, j * PAIR + DD)
                        csl = slice(j * PAIR + DD, (j + 1) * PAIR)
                        # ray 4g+pj (PE rows 0:64) -> partitions 0:48.
                        # lhsT widened to 64 cols (16 ctx cols as junk
                        # weights) so PSUM rows 48:64 are initialized for the
                        # block copy; rows 48:64 are dropped at host unpack.
                        nc.tensor.matmul(
                            out=pt[0:64, bank * BK:bank * BK + CC],
                            lhsT=dct[0:64, slice(j * PAIR, j * PAIR + 64)],
                            rhs=dct[0:64, csl],
                            start=True,
                            stop=True,
                        )
                        # ray 4g+2+pj (PE rows 64:128) -> partitions 48:96
                        nc.tensor.matmul(
                            out=pt[64:64 + DD, bank * BK:bank * BK + CC],
                            lhsT=dct[64:128, dsl],
                            rhs=dct[64:128, csl],
                            start=True,
                            stop=True,
                        )
                    if g % 2 == 1 or g == GPC - 1:
                        nb = 2 * (g % 2) + 2        # banks filled in this tile
                        blo = (g // 2) * 2          # first group in the tile
                        src = pt[0:112, 0:nb * BK].rearrange(
                            "p (b x) -> p b x", b=nb
                        )[:, :, 0:CC]
                        dst = stage[:, blo * 160:(g + 1) * 160].rearrange(
                            "p (b x) -> p b x", b=nb
                        )
                        if (g // 2) % 2 == 0:
                            nc.scalar.copy(out=dst, in_=src)
                        else:
                            nc.vector.tensor_copy(out=dst, in_=src)
                    # stream partials out on the idle GPSIMD SWDGE queue
                    # (final chunk on HWDGE: no ~1us SWDGE desc-gen in tail)
                    if (g + 1) % OGR == 0 or g == GPC - 1:
                        lo = (g // OGR) * OGR * 160
                        hi = (g + 1) * 160
                        oeng = nc.sync if g == GPC - 1 else nc.gpsimd
                        oeng.dma_start(out=part[:, lo:hi], in_=stage[:, lo:hi])
                g0 += GCH
    nc.compile()
    return nc


# Unpack map for L1 "part" output (112 partitions; rows 48:64 are junk):
#   parts 0:48   : [:, g, 0] = ray 4g+0, [:, g, 1] = ray 4g+1
#   parts 64:112 : [:, g, 0] = ray 4g+2, [:, g, 1] = ray 4g+3
def _unpack_l1(out_core):
    S = out_core.reshape(112, GPC, 2, CC)
    p = np.empty((RPC, DD, CC), out_core.dtype)
    p[0::4] = S[0:DD, :, 0].transpose(1, 0, 2)
    p[1::4] = S[0:DD, :, 1].transpose(1, 0, 2)
    p[2::4] = S[64:112, :, 0].transpose(1, 0, 2)
    p[3::4] = S[64:112, :, 1].transpose(1, 0, 2)
    return p


# ---------------------------------------------------------------- L2 builder
@functools.lru_cache(maxsize=8)
def _build_l2(K):
    """K: tuple of NSLOT ints -- tiles per PSUM row-slot (uniform across cores).

    Slots are grouped 4 to a PSUM bank (4 BEV rows = 512 fp32 columns); after a
    bank's 4 slots finish accumulating 80-wide partial sums, the bank is copied
    to SBUF and hit with one 1x1-conv matmul (80->128, BN scale folded into the
    weight on host), then BN-bias + ReLU straight out of conv PSUM into the
    bf16 output stage.
    """
    T_u = sum(K)
    nc = bacc.Bacc("TRN2", target_bir_lowering=False, debug=False, num_devices=NCORES)
    vals = nc.dram_tensor("vals", [128, T_u * CC], BF16, kind="ExternalInput")
    # merged metadata: cols 0:64 = iota as raw bf16 pairs, 64:64+T_u = idx,
    # col 64+T_u = BN bias, cols 65+T_u:129+T_u = folded conv weight (bf16 raw,
    # partitions 0:80)
    meta = nc.dram_tensor("meta", [128, 129 + T_u], F32, kind="ExternalInput")
    y = nc.dram_tensor("y", [OC, NSLOT * BW], BF16, kind="ExternalOutput")

    b1 = min(K[0], T_u)          # small first chunk so compute starts early
    rest = T_u - b1
    bnd = [0, b1, b1 + rest // 3, b1 + (2 * rest) // 3, T_u]     # tile chunks

    with tile.TileContext(nc) as tc:
        with (
            tc.tile_pool(name="consts", bufs=1) as const_pool,
            tc.tile_pool(name="vals", bufs=4) as vals_pool,
            tc.tile_pool(name="oh", bufs=20) as oh_pool,
            tc.tile_pool(name="bev", bufs=2) as bev_pool,
            tc.tile_pool(name="yst", bufs=1) as yst_pool,
            tc.tile_pool(name="psA", bufs=2, space="PSUM") as psA_pool,
            tc.tile_pool(name="psC", bufs=2, space="PSUM") as psC_pool,
        ):
            # prewarm the Activation function table at t=0
            warm = const_pool.tile([128, 2], BF16)
            nc.gpsimd.memset(warm[:, 0:1], 0)
            nc.scalar.activation(
                out=warm[:, 1:2], in_=warm[:, 0:1],
                func=mybir.ActivationFunctionType.Relu,
            )
            meta_t = const_pool.tile([128, 129 + T_u], F32)
            nc.sync.dma_start(out=meta_t[:], in_=meta[:])
            iota_t = meta_t[:, 0:64].bitcast(BF16)          # (128, 128) iota
            idx_t = meta_t[:, 64:64 + T_u]
            bias_ap = meta_t[:, 64 + T_u:65 + T_u]
            wS_t = meta_t[0:CC, 65 + T_u:129 + T_u].bitcast(BF16)   # (80, 128)

            vt = []
            for chk in range(4):
                t = vals_pool.tile([128, (bnd[chk + 1] - bnd[chk]) * CC], BF16)
                eng = nc.sync
                eng.dma_start(
                    out=t[:], in_=vals[:, bnd[chk] * CC:bnd[chk + 1] * CC]
                )
                vt.append(t)

            def val_slice(tf):
                chk = next(i for i in range(4) if bnd[i] <= tf < bnd[i + 1])
                lo = (tf - bnd[chk]) * CC
                return vt[chk][:, lo:lo + CC]

            yst = yst_pool.tile([OC, NSLOT * BW], BF16)
            tf = 0
            for q in range(4):                   # 4 slots -> 1 PSUM bank
                ps = psA_pool.tile([128, 512], F32, space="PSUM")
                for i in range(4):
                    s = 4 * q + i
                    for k in range(K[s]):
                        oh = oh_pool.tile([128, 128], BF16)
                        # spread some one-hot builds onto the GPSIMD engine
                        oheng = nc.gpsimd if (tf % 3 == 2) else nc.vector
                        oheng.tensor_scalar(
                            out=oh[:],
                            in0=iota_t,
                            scalar1=idx_t[:, tf:tf + 1],
                            scalar2=None,
                            op0=mybir.AluOpType.is_equal,
                        )
                        nc.tensor.matmul(
                            out=ps[0:CC, i * BW:(i + 1) * BW],
                            lhsT=val_slice(tf),
                            rhs=oh[:],
                            start=(k == 0),
                            stop=(k == K[s] - 1),
                        )
                        tf += 1
                # drain bank: copy -> conv -> BN bias + ReLU, each stage on
                # its own engine so the 4 bank drains pipeline cleanly
                bev = bev_pool.tile([CC, 512], BF16)
                if q % 2 == 0:
                    nc.vector.tensor_copy(out=bev[:], in_=ps[0:CC, :])
                else:
                    nc.scalar.copy(out=bev[:], in_=ps[0:CC, :])
                psc = psC_pool.tile([OC, 512], F32, space="PSUM")
                nc.tensor.matmul(
                    out=psc[:], lhsT=wS_t, rhs=bev[:], start=True, stop=True
                )
                nc.scalar.activation(
                    out=yst[:, q * 512:(q + 1) * 512],
                    in_=psc[:],
                    func=mybir.ActivationFunctionType.Relu,
                    bias=bias_ap,
                    scale=1.0,
                )
                yeng = nc.sync
                yeng.dma_start(
                    out=y[:, q * 512:(q + 1) * 512],
                    in_=yst[:, q * 512:(q + 1) * 512],
                )
    nc.compile()
    return nc


# ---------------------------------------------------------------- host plan
def _compute_bins(intrinsics, cam2ego):
    """Mirror the reference's index math exactly (same jnp ops, same backend)
    so floor() results match bit-for-bit, then reduce over the h axis."""
    import jax.numpy as jnp

    intrinsics = jnp.asarray(intrinsics)
    cam2ego = jnp.asarray(cam2ego)
    u = ((jnp.arange(WF, dtype=jnp.float32) + 0.5) * STRIDE)[None, None, None, None, :]
    v = ((jnp.arange(HF, dtype=jnp.float32) + 0.5) * STRIDE)[None, None, None, :, None]
    Z = jnp.linspace(Z_MIN, Z_MAX, DD, dtype=jnp.float32)[None, None, :, None, None]

    fx = intrinsics[:, :, 0, 0][:, :, None, None, None]
    fy = intrinsics[:, :, 1, 1][:, :, None, None, None]
    cx = intrinsics[:, :, 0, 2][:, :, None, None, None]
    cy = intrinsics[:, :, 1, 2][:, :, None, None, None]

    Xc = (u - cx) / fx * Z
    Yc = (v - cy) / fy * Z
    Zc = jnp.broadcast_to(Z, Xc.shape)

    T = cam2ego[:, :, None, None, None]
    x_e = T[..., 0, 0] * Xc + T[..., 0, 1] * Yc + T[..., 0, 2] * Zc + T[..., 0, 3]
    y_e = T[..., 1, 0] * Xc + T[..., 1, 1] * Yc + T[..., 1, 2] * Zc + T[..., 1, 3]

    mx = (PC[3] - PC[0]) / BW
    my = (PC[4] - PC[1]) / BH
    ix = jnp.floor((x_e - PC[0]) / mx).astype(jnp.int32)
    iy = jnp.floor((y_e - PC[1]) / my).astype(jnp.int32)
    valid = (ix >= 0) & (ix < BW) & (iy >= 0) & (iy < BH)

    ix = np.asarray(ix)[0]
    iy = np.asarray(iy)[0]
    valid = np.asarray(valid)[0]
    # h-independence (holds for z-yaw-only rigs; required by this kernel)
    assert (ix == ix[:, :, :1, :]).all() and (iy == iy[:, :, :1, :]).all() and (
        valid == valid[:, :, :1, :]
    ).all(), "BEV bin depends on image row; kernel assumes z-yaw-only rig"
    return ix[:, :, 0, :], iy[:, :, 0, :], valid[:, :, 0, :]   # (N, D, W)


def _plan(intrinsics, cam2ego):
    ix, iy, valid = _compute_bins(intrinsics, cam2ego)
    # global point id = ray*DD + d, ray = n*WF + w
    ixr = ix.transpose(0, 2, 1).reshape(-1)      # (n, w, d) flattened
    iyr = iy.transpose(0, 2, 1).reshape(-1)
    vr = valid.transpose(0, 2, 1).reshape(-1)
    pid = np.arange(RAYS * DD, dtype=np.int64)

    vpid = pid[vr]
    vrow = iyr[vr].astype(np.int64)
    vcol = ixr[vr].astype(np.int64)

    # group points by BEV row
    order = np.argsort(vrow, kind="stable")
    vpid, vrow, vcol = vpid[order], vrow[order], vcol[order]
    rowcnt = np.bincount(vrow, minlength=BH)
    rowstart = np.concatenate([[0], np.cumsum(rowcnt)])
    tiles_per_row = np.maximum((rowcnt + 127) // 128, rowcnt > 0).astype(int)

    # snake-deal rows to cores by descending tile count -> 16 rows per core
    rorder = np.argsort(-tiles_per_row, kind="stable")
    core_rows = [[] for _ in range(NCORES)]
    for i, r in enumerate(rorder):
        rnd, pos = divmod(i, NCORES)
        c = pos if rnd % 2 == 0 else NCORES - 1 - pos
        core_rows[c].append(int(r))
    # per-core: rows sorted by tile count desc -> slot s
    for c in range(NCORES):
        core_rows[c].sort(key=lambda r: -tiles_per_row[r])
    K = tuple(
        int(max(tiles_per_row[core_rows[c][s]] for c in range(NCORES)))
        for s in range(NSLOT)
    )
    return dict(
        K=K,
        core_rows=core_rows,
        rowstart=rowstart,
        rowcnt=rowcnt,
        vpid=vpid,
        vcol=vcol,
    )


# ---------------------------------------------------------------- main entry
def _l1_inputs(depth_prob, context):
    dt = _np_bf16()
    dT = depth_prob[0].transpose(2, 0, 3, 1).reshape(HF, RAYS, DD)  # h, ray, d
    cT = context[0].transpose(2, 0, 3, 1).reshape(HF, RAYS, CC)     # h, ray, c
    maps = []
    for c in range(NCORES):
        sl = slice(c * RPC, (c + 1) * RPC)
        d4 = dT[:, sl].reshape(HF, GPC, 2, 2, DD)    # h, g, half, pj, d
        c4 = cT[:, sl].reshape(HF, GPC, 2, 2, CC)
        dc = np.concatenate([d4, c4], axis=-1)       # h, g, half, pj, 128
        dc = dc.transpose(2, 0, 1, 3, 4).reshape(128, GPC * 2 * PAIR)
        maps.append({"dc_in": np.ascontiguousarray(dc).astype(dt)})
    return maps


def _l2_inputs(plan, part_all, w_proj, b_proj, bn_gamma, bn_beta, bn_mean, bn_var):
    dt = _np_bf16()
    K = plan["K"]
    T_u = sum(K)
    scale = (bn_gamma / np.sqrt(bn_var + BN_EPS)).astype(np.float32)
    bias = ((b_proj - bn_mean) * scale + bn_beta).astype(np.float32)
    # fold BN scale into the conv weight; device conv runs after the scatter.
    wS = (w_proj * scale[:, None]).astype(np.float32)        # (OC, CC)
    wS_raw = np.zeros((128, 64), np.float32)
    wS_raw[0:CC] = np.ascontiguousarray(wS.T.astype(dt)).view(np.float32)
    iota_raw = np.ascontiguousarray(
        np.broadcast_to(np.arange(128, dtype=np.float32).astype(dt), (128, 128))
    ).view(np.float32)                                       # (128, 64) raw bits

    rowstart, vpid, vcol = plan["rowstart"], plan["vpid"], plan["vcol"]
    maps = []
    for c in range(NCORES):
        vals = np.zeros((128, T_u, CC), np.float32)
        idx = np.full((128, T_u), -1.0, np.float32)
        tf = 0
        for s in range(NSLOT):
            r = plan["core_rows"][c][s]
            lo, hi = rowstart[r], rowstart[r + 1]
            pids = vpid[lo:hi]
            cols = vcol[lo:hi]
            for k in range(K[s]):
                seg = slice(k * 128, min((k + 1) * 128, hi - lo))
                n = max(0, seg.stop - seg.start)
                if n > 0:
                    vals[:n, tf] = part_all[pids[seg]]
                    idx[:n, tf] = cols[seg]
                tf += 1
        meta = np.concatenate(
            [iota_raw, idx, bias[:, None], wS_raw], axis=1
        ).astype(np.float32)
        maps.append(
            {
                "vals": vals.reshape(128, -1).astype(dt),
                "meta": meta,
            }
        )
    return maps


def kernel(**inputs) -> np.ndarray:
    depth_prob = np.asarray(inputs["depth_prob"], np.float32)
    context = np.asarray(inputs["context"], np.float32)
    intrinsics = np.asarray(inputs["intrinsics"], np.float32)
    cam2ego = np.asarray(inputs["cam2ego"], np.float32)

    plan = _plan(intrinsics, cam2ego)
    nc1 = _build_l1()
    l1_maps = _l1_inputs(depth_prob, context)
    res1 = run_bass_kernel_spmd(nc1, l1_maps, list(range(NCORES))).results

    part_all = np.concatenate(
        [_unpack_l1(res1[c]["part"]) for c in range(NCORES)], axis=0
    ).reshape(RAYS * DD, CC).astype(np.float32)

    nc2 = _build_l2(plan["K"])
    l2_maps = _l2_inputs(
        plan,
        part_all,
        np.asarray(inputs["w_proj"], np.float32),
        np.asarray(inputs["b_proj"], np.float32),
        np.asarray(inputs["bn_gamma"], np.float32),
        np.asarray(inputs["bn_beta"], np.float32),
        np.asarray(inputs["bn_mean"], np.float32),
        np.asarray(inputs["bn_var"], np.float32),
    )
    res2 = run_bass_kernel_spmd(nc2, l2_maps, list(range(NCORES))).results

    y = np.empty((1, OC, BH, BW), np.float32)
    for c in range(NCORES):
        yc = np.asarray(res2[c]["y"], np.float32)      # (OC, NSLOT*BW)
        for s in range(NSLOT):
            r = plan["core_rows"][c][s]
            y[0, :, r, :] = yc[:, s * BW:(s + 1) * BW]
    return y

